# revision 4
# baseline (speedup 1.0000x reference)
"""BitLinear on 8 Trainium2 NeuronCores, token-parallel, fp8 DoubleRow matmul.

Math notes:
  activation_quant: q = round(xn * s_t), s_t = 127/(amax(|xn|)+eps); xq = q/s_t.
  weight_quant:     w3 = clip(round(w*s_w), -1, 1), s_w = 1/(mean|w|+eps).
  out = xq @ wq.T = (q @ w3.T) * rowscale,  rowscale = (amax_n+eps)*(mean|w|+eps)/127.

  fp8 DoubleRow trick: q = 16*qh + ql with qh = RNE(q/16) in [-8,8],
  ql = q - 16*qh in [-8,8]. qh16 = 16*qh in {-128..128 step 16} and ql are both
  exactly representable in fp8e4m3, as is w3 in {-1,0,1}. One DoubleRow matmul
  contracts two 128-deep k-slots per cycle pair:
      psum += qh16_slot.T @ w3 + ql_slot.T @ w3  =  (q @ w3) exactly,
  at 2x the bf16 MAC rate. All accumulation is integer-valued f32 (< 2^19), so
  the quantization side is exact; rounding errors only enter via s_w/s_t ulps.

  Rounds are RNE (matches jnp.round) via magic constants: f32: +/-1.5*2^23
  (weights), fp16: bias 1536 = 1.5*2^10 (activation round, |v|<=127),
  bf16: 192 = 1.5*2^7 (qh round, |v|<=8).

Per-core program (Tc=2048 tokens, D=2048, F=8192):
  stage A: stream wA (this core's F/8 slice), abs-sum -> mean|w| via AllReduce
           + ones-matmul partition broadcast -> swinv.
  stage X (per 128-token tile): stream x rows, ACT square+accum -> s1,
           DVE absmax -> s2, small-vector beta chain, ACT q16 = fp16(1536+x*beta),
           DVE q = q16-1536 (bf16), qh_t = q/16+192 (bf16),
           qh16 = (qh_t-192)*16 (bf16), ql = q - qh16 (bf16);
           PE-transpose qh16/ql 128x128 blocks -> PSUM, ACT-drain (cast fp8e4)
           into the stationary pair slab qpair[P, nD, 2, Tc].
  stage M: per 1024-wide feature group: DVE w round-mult (C+RNE(w*s_w)),
           DVE C-space clamp, ACT -C + fp8 cast -> w3[j]; then per token tile
           one [128,1024] 2-bank PSUM DoubleRow accumulation (moving = w3
           broadcast to both k-slots), ACT scaled drain (bf16), DMA out.
  Output is bf16 on-device, upcast to f32 on host (rel err ~1e-3 << 2e-2).
"""
import os
import numpy as np
import concourse.bacc as bacc
import concourse.tile as tile
import concourse.mybir as mybir
from concourse import masks
from concourse.bass_utils import run_bass_kernel_spmd

Alu = mybir.AluOpType
Act = mybir.ActivationFunctionType
PerfMode = mybir.MatmulPerfMode
F32 = mybir.dt.float32
F16 = mybir.dt.float16
BF16 = mybir.dt.bfloat16
FP8 = mybir.dt.float8e4

C = 1.5 * 2.0 ** 23    # f32 RNE magic
CH = 1536.0            # fp16 RNE magic (1.5*2^10), |v| <= 511
CB = 192.0             # bf16 RNE magic (1.5*2^7),  |v| <= 63
EPS_NORM = 1e-6
EPS_ACT = 1e-5
EPS_W = 1e-5
P = 128
FCH = 512              # one PSUM bank of f32
FG = 1024              # feature group (2 PSUM banks)

NCORES = 8


def _strip_redundant_ldweights(nc):
    """Remove InstLdweights that reload the exact AP the PE already holds and
    carry no sync. Runs after tile scheduling, before compile."""
    n = 0
    for blk in nc.m.functions[0].blocks:
        keep = []
        prev_key = None
        for inst in blk.instructions:
            nm = type(inst).__name__
            if nm == "InstMatmult":
                if getattr(inst, "ldweights", None) is not False:
                    prev_key = None
                keep.append(inst)
                continue
            if nm == "InstLdweights":
                key = str(inst.ins[0])
                si = inst.sync_info
                clean = si is None or (not si.on_wait and not si.on_update)
                if key == prev_key and clean:
                    n += 1
                    continue
                prev_key = key
                keep.append(inst)
                continue
            prev_key = None
            keep.append(inst)
        blk.instructions[:] = keep
    return n


def build_program(Tc, D, F, n_devices=NCORES, g_is_ones=True, reps=1,
                  out_f32=False, skip_cc=False):
    nT = Tc // P
    nD = D // P
    FA = F // n_devices if n_devices > 1 else F
    nG = F // FG
    nCh = FG // FCH
    ODT = F32 if out_f32 else BF16

    nc = bacc.Bacc("TRN2", num_devices=n_devices)
    x = nc.dram_tensor("x", [Tc, D], F32, kind="ExternalInput")
    wT = nc.dram_tensor("wT", [D, F], F32, kind="ExternalInput")
    g = nc.dram_tensor("g", [1, D], F32, kind="ExternalInput")
    wA = nc.dram_tensor("wA", [D, FA], F32, kind="ExternalInput")
    out = nc.dram_tensor("out", [Tc, F], ODT, kind="ExternalOutput")
    done = nc.dram_tensor("done", [1, 8], F32, kind="ExternalOutput")
    cc_in = nc.dram_tensor("cc_in", [P, 1], F32)
    cc_out = nc.dram_tensor("cc_out", [P, 1], F32)

    with tile.TileContext(nc) as tc:
        with tc.tile_pool(name="const", bufs=1) as const_pool, \
             tc.tile_pool(name="stats", bufs=1) as stats_pool, \
             tc.tile_pool(name="qres", bufs=1) as qres_pool, \
             tc.tile_pool(name="wa", bufs=2) as wa_pool, \
             tc.tile_pool(name="xi", bufs=2) as xi_pool, \
             tc.tile_pool(name="qrow", bufs=2) as qrow_pool, \
             tc.tile_pool(name="wb", bufs=3) as wb_pool, \
             tc.tile_pool(name="wr", bufs=2) as wr_pool, \
             tc.tile_pool(name="w3", bufs=2) as w3_pool, \
             tc.tile_pool(name="osb", bufs=2) as osb_pool, \
             tc.tile_pool(name="ps_small", bufs=1, space="PSUM") as pss, \
             tc.tile_pool(name="ps_tr", bufs=1, space="PSUM") as pst, \
             tc.tile_pool(name="ps_out", bufs=2, space="PSUM") as pso:

            def _kbody():
                # ---- constants ----
                identb = const_pool.tile([P, P], BF16)
                masks.make_identity(nc, identb[:])
                ones_col = const_pool.tile([P, P], F32)
                nc.vector.memset(ones_col[:], 1.0)
                if not g_is_ones:
                    ones_row = const_pool.tile([1, P], F32)
                    nc.vector.memset(ones_row[:], 1.0)
                    g_bc = const_pool.tile([P, D], F32)
                    g_row = const_pool.tile([1, D], F32)
                    nc.sync.dma_start(g_row[:], g.ap())
                    for st in range(0, D, FCH):
                        pgb = pss.tile([P, FCH], F32, tag="pgb")
                        nc.tensor.matmul(pgb[:], ones_row[:], g_row[:, st:st + FCH],
                                         start=True, stop=True)
                        nc.scalar.activation(g_bc[:, st:st + FCH], pgb[:],
                                             Act.Copy, bias=0.0, scale=1.0)

                # persistent per-token stats (column layout [P, nT])
                wmeane = stats_pool.tile([P, 1], F32)
                swinv = stats_pool.tile([P, 1], F32)
                rowscale = stats_pool.tile([P, nT], F32)
                s1 = stats_pool.tile([P, nT], F32)
                s2 = stats_pool.tile([P, nT], F32)
                rinv = stats_pool.tile([P, nT], F32)
                aei = stats_pool.tile([P, nT], F32)
                beta = stats_pool.tile([P, nT], F32)
                # stationary pair slab: [d-part, j, slot(qh16|ql), token]
                qpair = qres_pool.tile([P, nD, 2, Tc], FP8)

                # ---- stage A: mean|w| over this core's F/8 slice ----
                acc_a = stats_pool.tile([P, nD], F32)
                for j in range(nD):
                    wa = wa_pool.tile([P, FA], F32)
                    nc.sync.dma_start(wa[:], wA.ap()[j * P:(j + 1) * P, :])
                    wascr = wr_pool.tile([P, max(FG, FA)], F32, tag="wr")
                    nc.scalar.activation(wascr[:, :FA], wa[:], Act.Abs, bias=0.0,
                                         scale=1.0, accum_out=acc_a[:, j:j + 1])
                acc1 = stats_pool.tile([P, 1], F32)
                nc.vector.tensor_reduce(acc1[:], acc_a[:], axis=mybir.AxisListType.X,
                                        op=Alu.add)
                if n_devices > 1 and reps == 1 and not skip_cc:
                    nc.sync.dma_start(cc_in.ap(), acc1[:])
                    nc.gpsimd.collective_compute(
                        "AllReduce", Alu.add,
                        replica_groups=[list(range(n_devices))],
                        ins=[cc_in.ap().opt()],
                        outs=[cc_out.ap().opt()],
                    )
                    ccred = stats_pool.tile([P, 1], F32)
                    nc.sync.dma_start(ccred[:], cc_out.ap())
                else:
                    ccred = acc1
                ptot = pss.tile([P, 1], F32, tag="ptot")
                nc.tensor.matmul(ptot[:], ones_col[:], ccred[:], start=True, stop=True)
                nc.scalar.activation(wmeane[:], ptot[:], Act.Copy,
                                     bias=float(EPS_W), scale=1.0 / float(D * F))
                nc.vector.reciprocal(swinv[:], wmeane[:])

                # ---- stage M helpers ----
                def _w3_prep(fg):
                    w3 = []
                    for j in range(nD):
                        wb = wb_pool.tile([P, FG], F32)
                        nc.sync.dma_start(wb[:], wT.ap()[j * P:(j + 1) * P,
                                                         fg * FG:(fg + 1) * FG])
                        wr = wr_pool.tile([P, FG], F32, tag="wr")
                        # wr = C + RNE(w*s_w)
                        nc.vector.tensor_scalar(wr[:], wb[:], swinv[:, 0:1], C,
                                                op0=Alu.mult, op1=Alu.add)
                        # clamp in C-space: C + clip(RNE, -1, 1)
                        nc.vector.tensor_scalar(wr[:], wr[:], C - 1.0, C + 1.0,
                                                op0=Alu.max, op1=Alu.min)
                        w3j = w3_pool.tile([P, FG], FP8, tag=f"w3_{j}")
                        nc.scalar.activation(w3j[:], wr[:], Act.Copy,
                                             bias=-C, scale=1.0)
                        w3.append(w3j)
                    return w3

                def _mm_block(fg, ti, w3):
                    pout = pso.tile([P, FG], F32)
                    for j in range(nD):
                        stat = qpair[:, j, :, ti * P:(ti + 1) * P]
                        for fc in range(nCh):
                            mov = w3[j][:, fc * FCH:(fc + 1) * FCH] \
                                .unsqueeze(1).broadcast_to([P, 2, FCH])
                            nc.tensor.matmul(
                                pout[:, fc * FCH:(fc + 1) * FCH],
                                stat, mov,
                                start=(j == 0), stop=(j == nD - 1),
                                perf_mode=PerfMode.DoubleRow)
                    ost = osb_pool.tile([P, FG], ODT)
                    nc.scalar.activation(ost[:], pout[:], Act.Copy, bias=0.0,
                                         scale=rowscale[:, ti:ti + 1])
                    nc.sync.dma_start(out.ap()[ti * P:(ti + 1) * P,
                                               fg * FG:(fg + 1) * FG], ost[:])

                # fg0 weights prepared up front so its matmul sweep can
                # interleave with the stage-X per-tile pipeline (the PE is
                # in-order: without this it would idle until all transposes
                # are produced before starting stage M)
                w3_0 = _w3_prep(0)

                # ---- stage X: per-token-tile stats + quant + transpose ----
                for i in range(nT):
                    xi = xi_pool.tile([P, D], F32, tag="xi")
                    nc.sync.dma_start(xi[:], x.ap()[i * P:(i + 1) * P, :])
                    if g_is_ones:
                        xuse = xi
                    else:
                        xg = xi_pool.tile([P, D], F32, tag="xg")
                        nc.vector.tensor_tensor(xg[:], xi[:], g_bc[:], op=Alu.mult)
                        xuse = xg
                    nc.vector.tensor_reduce(s2[:, i:i + 1], xuse[:],
                                            axis=mybir.AxisListType.X,
                                            op=Alu.max, apply_absolute_value=True)
                    sqscr = qrow_pool.tile([P, D], BF16, tag="sqscr")
                    nc.scalar.activation(sqscr[:], xi[:], Act.Square, bias=0.0,
                                         scale=1.0, accum_out=s1[:, i:i + 1])
                    # beta chain on [P,1] columns
                    si, zi = s1[:, i:i + 1], s2[:, i:i + 1]
                    ri, ai, bi = rinv[:, i:i + 1], aei[:, i:i + 1], beta[:, i:i + 1]
                    nc.vector.tensor_scalar(si, si, 1.0 / float(D), float(EPS_NORM),
                                            op0=Alu.mult, op1=Alu.add)
                    nc.scalar.activation(si, si, Act.Sqrt, bias=0.0, scale=1.0)
                    nc.vector.reciprocal(ri, si)           # 1/rms
                    nc.vector.tensor_scalar(ai, zi, ri, float(EPS_ACT),
                                            op0=Alu.mult, op1=Alu.add)  # ae
                    nc.vector.tensor_scalar(rowscale[:, i:i + 1], ai,
                                            wmeane[:, 0:1], 1.0 / 127.0,
                                            op0=Alu.mult, op1=Alu.mult)
                    nc.vector.reciprocal(ai, ai)           # 1/ae
                    nc.vector.tensor_scalar(bi, ai, ri, 127.0,
                                            op0=Alu.mult, op1=Alu.mult)  # beta
                    # quant rows: q16 = fp16(CH + x*beta) = CH + q
                    q16 = qrow_pool.tile([P, D], F16, tag="q16")
                    nc.scalar.activation(q16[:], xuse[:], Act.Copy,
                                         bias=CH, scale=bi)
                    qb = qrow_pool.tile([P, D], BF16, tag="qb")
                    nc.vector.tensor_scalar(qb[:], q16[:], CH, None,
                                            op0=Alu.subtract)
                    qht = qrow_pool.tile([P, D], BF16, tag="qht")
                    nc.vector.tensor_scalar(qht[:], qb[:], 1.0 / 16.0, CB,
                                            op0=Alu.mult, op1=Alu.add)
                    qh16 = qrow_pool.tile([P, D], BF16, tag="qh16")
                    nc.vector.tensor_scalar(qh16[:], qht[:], CB, 16.0,
                                            op0=Alu.subtract, op1=Alu.mult)
                    ql = qrow_pool.tile([P, D], BF16, tag="ql")
                    nc.vector.tensor_tensor(ql[:], qb[:], qh16[:], op=Alu.subtract)
                    # transpose 128x128 blocks into PSUM (8 blocks per bank),
                    # drain with fp8 cast into qpair slots
                    for half in range(2):
                        j0 = half * (nD // 2)
                        ph = pst.tile([P, nD // 2, P], BF16, tag="ph")
                        for jj in range(nD // 2):
                            nc.tensor.transpose(
                                ph[:, jj, :],
                                qh16[:, (j0 + jj) * P:(j0 + jj + 1) * P],
                                identb[:])
                        nc.scalar.activation(
                            qpair[:, j0:j0 + nD // 2, 0, i * P:(i + 1) * P],
                            ph[:], Act.Copy, bias=0.0, scale=1.0)
                        pl = pst.tile([P, nD // 2, P], BF16, tag="pl")
                        for jj in range(nD // 2):
                            nc.tensor.transpose(
                                pl[:, jj, :],
                                ql[:, (j0 + jj) * P:(j0 + jj + 1) * P],
                                identb[:])
                        nc.scalar.activation(
                            qpair[:, j0:j0 + nD // 2, 1, i * P:(i + 1) * P],
                            pl[:], Act.Copy, bias=0.0, scale=1.0)
                    # fg0 matmuls for this tile, interleaved into the PE
                    # stream right after the tile's transposes
                    _mm_block(0, i, w3_0)

                # ---- stage M: remaining feature groups ----
                for fg in range(1, nG):
                    w3 = _w3_prep(fg)
                    for ti in range(nT):
                        _mm_block(fg, ti, w3)

                dsb = stats_pool.tile([1, 8], F32)
                nc.vector.memset(dsb[:], 1.0)
                nc.sync.dma_start(done.ap(), dsb[:])

            if reps == 1:
                _kbody()
            else:
                with tc.For_i(0, reps, 1):
                    _kbody()

    if not os.environ.get("BITLIN_NO_STRIP"):
        _strip_redundant_ldweights(nc)
    nc.compile()
    return nc


_prog_cache = {}


def _get_program(Tc, D, F, g_is_ones=True, out_f32=False):
    key = (Tc, D, F, g_is_ones, out_f32)
    if key not in _prog_cache:
        _prog_cache[key] = build_program(Tc, D, F, g_is_ones=g_is_ones,
                                         out_f32=out_f32)
    return _prog_cache[key]


def make_in_maps(x, norm_weight, weight):
    B, S, D = x.shape
    F = weight.shape[0]
    T = B * S
    Tc = T // NCORES
    xf = np.ascontiguousarray(x.reshape(T, D), dtype=np.float32)
    wTv = np.ascontiguousarray(weight.T).astype(np.float32, copy=False)
    gv = np.ascontiguousarray(norm_weight.reshape(1, D), dtype=np.float32)
    FA = F // NCORES
    in_maps = []
    for c in range(NCORES):
        in_maps.append({
            "x": xf[c * Tc:(c + 1) * Tc],
            "wT": wTv,
            "wA": np.ascontiguousarray(wTv[:, c * FA:(c + 1) * FA]),
            "g": gv,
        })
    return in_maps, (B, S, T, Tc, D, F)


def kernel(x, norm_weight, weight):
    x = np.asarray(x)
    norm_weight = np.asarray(norm_weight)
    weight = np.asarray(weight)
    in_maps, (B, S, T, Tc, D, F) = make_in_maps(x, norm_weight, weight)
    out_f32 = bool(os.environ.get("BITLIN_OUT_F32"))
    nc = _get_program(Tc, D, F, g_is_ones=bool(np.all(norm_weight == 1.0)),
                      out_f32=out_f32)
    res = None
    last_err = None
    for _ in range(3):
        try:
            res = run_bass_kernel_spmd(nc, in_maps, core_ids=list(range(NCORES)))
            break
        except Exception as e:
            last_err = e
    if res is None:
        raise last_err
    outp = np.concatenate([np.asarray(res.results[c]["out"], dtype=np.float32)
                           for c in range(NCORES)], axis=0)
    return np.ascontiguousarray(outp.reshape(B, S, F))


# revision 5
# speedup vs baseline: 1.0549x; 1.0549x over previous
"""BitLinear on 8 Trainium2 NeuronCores, token-parallel, fp8 DoubleRow matmul.

Math notes:
  activation_quant: q = round(xn * s_t), s_t = 127/(amax(|xn|)+eps); xq = q/s_t.
  weight_quant:     w3 = clip(round(w*s_w), -1, 1), s_w = 1/(mean|w|+eps).
  out = xq @ wq.T = (q @ w3.T) * rowscale,  rowscale = (amax_n+eps)*(mean|w|+eps)/127.

  fp8 DoubleRow trick: q = 16*qh + ql with qh = RNE(q/16) in [-8,8],
  ql = q - 16*qh in [-8,8]. qh16 = 16*qh in {-128..128 step 16} and ql are both
  exactly representable in fp8e4m3, as is w3 in {-1,0,1}. One DoubleRow matmul
  contracts two 128-deep k-slots per cycle pair:
      psum += qh16_slot.T @ w3 + ql_slot.T @ w3  =  (q @ w3) exactly,
  at 2x the bf16 MAC rate. All accumulation is integer-valued f32 (< 2^19), so
  the quantization side is exact; rounding errors only enter via s_w/s_t ulps.

  Rounds are RNE (matches jnp.round) via magic constants: f32: +/-1.5*2^23
  (weights), fp16: bias 1536 = 1.5*2^10 (activation round, |v|<=127),
  bf16: 192 = 1.5*2^7 (qh round, |v|<=8).

Per-core program (Tc=2048 tokens, D=2048, F=8192):
  stage A: stream wA (this core's F/8 slice), abs-sum -> mean|w| via AllReduce
           + ones-matmul partition broadcast -> swinv.
  stage X (per 128-token tile): stream x rows, ACT square+accum -> s1,
           DVE absmax -> s2, small-vector beta chain, ACT q16 = fp16(1536+x*beta),
           DVE q = q16-1536 (bf16), qh_t = q/16+192 (bf16),
           qh16 = (qh_t-192)*16 (bf16), ql = q - qh16 (bf16);
           PE-transpose qh16/ql 128x128 blocks -> PSUM, ACT-drain (cast fp8e4)
           into the stationary pair slab qpair[P, nD, 2, Tc].
  stage M: per 1024-wide feature group: DVE w round-mult (C+RNE(w*s_w)),
           DVE C-space clamp, ACT -C + fp8 cast -> w3[j]; then per token tile
           one [128,1024] 2-bank PSUM DoubleRow accumulation (moving = w3
           broadcast to both k-slots), ACT scaled drain (bf16), DMA out.
  Output is bf16 on-device, upcast to f32 on host (rel err ~1e-3 << 2e-2).
"""
import os
import numpy as np
import concourse.bacc as bacc
import concourse.tile as tile
import concourse.mybir as mybir
from concourse import masks
from concourse.bass_utils import run_bass_kernel_spmd

Alu = mybir.AluOpType
Act = mybir.ActivationFunctionType
PerfMode = mybir.MatmulPerfMode
F32 = mybir.dt.float32
F16 = mybir.dt.float16
BF16 = mybir.dt.bfloat16
FP8 = mybir.dt.float8e4

C = 1.5 * 2.0 ** 23    # f32 RNE magic
CH = 1536.0            # fp16 RNE magic (1.5*2^10), |v| <= 511
CB = 192.0             # bf16 RNE magic (1.5*2^7),  |v| <= 63
EPS_NORM = 1e-6
EPS_ACT = 1e-5
EPS_W = 1e-5
P = 128
FCH = 512              # one PSUM bank of f32
FG = 1024              # feature group (2 PSUM banks)

NCORES = 8


def _strip_redundant_ldweights(nc):
    """Remove InstLdweights that reload the exact AP the PE already holds and
    carry no sync. Runs after tile scheduling, before compile."""
    n = 0
    for blk in nc.m.functions[0].blocks:
        keep = []
        prev_key = None
        for inst in blk.instructions:
            nm = type(inst).__name__
            if nm == "InstMatmult":
                if getattr(inst, "ldweights", None) is not False:
                    prev_key = None
                keep.append(inst)
                continue
            if nm == "InstLdweights":
                key = str(inst.ins[0])
                si = inst.sync_info
                clean = si is None or (not si.on_wait and not si.on_update)
                if key == prev_key and clean:
                    n += 1
                    continue
                prev_key = key
                keep.append(inst)
                continue
            prev_key = None
            keep.append(inst)
        blk.instructions[:] = keep
    return n


def build_program(Tc, D, F, n_devices=NCORES, g_is_ones=True, reps=1,
                  out_f32=False, skip_cc=False):
    nT = Tc // P
    nD = D // P
    FA = F // n_devices if n_devices > 1 else F
    nG = F // FG
    nCh = FG // FCH
    ODT = F32 if out_f32 else BF16

    nc = bacc.Bacc("TRN2", num_devices=n_devices)
    x = nc.dram_tensor("x", [Tc, D], F32, kind="ExternalInput")
    wT = nc.dram_tensor("wT", [D, F], F32, kind="ExternalInput")
    g = nc.dram_tensor("g", [1, D], F32, kind="ExternalInput")
    wA = nc.dram_tensor("wA", [D, FA], F32, kind="ExternalInput")
    out = nc.dram_tensor("out", [Tc, F], ODT, kind="ExternalOutput")
    done = nc.dram_tensor("done", [1, 8], F32, kind="ExternalOutput")
    cc_in = nc.dram_tensor("cc_in", [P, 1], F32)
    cc_out = nc.dram_tensor("cc_out", [P, 1], F32)

    with tile.TileContext(nc) as tc:
        with tc.tile_pool(name="const", bufs=1) as const_pool, \
             tc.tile_pool(name="stats", bufs=1) as stats_pool, \
             tc.tile_pool(name="qres", bufs=1) as qres_pool, \
             tc.tile_pool(name="wa", bufs=2) as wa_pool, \
             tc.tile_pool(name="xi", bufs=2) as xi_pool, \
             tc.tile_pool(name="qrow", bufs=2) as qrow_pool, \
             tc.tile_pool(name="wb", bufs=3) as wb_pool, \
             tc.tile_pool(name="wr", bufs=2) as wr_pool, \
             tc.tile_pool(name="w3", bufs=2) as w3_pool, \
             tc.tile_pool(name="osb", bufs=2) as osb_pool, \
             tc.tile_pool(name="ps_small", bufs=1, space="PSUM") as pss, \
             tc.tile_pool(name="ps_tr", bufs=1, space="PSUM") as pst, \
             tc.tile_pool(name="ps_out", bufs=2, space="PSUM") as pso:

            def _kbody():
                # ---- constants ----
                identb = const_pool.tile([P, P], BF16)
                masks.make_identity(nc, identb[:])
                ones_col = const_pool.tile([P, P], F32)
                nc.vector.memset(ones_col[:], 1.0)
                if not g_is_ones:
                    ones_row = const_pool.tile([1, P], F32)
                    nc.vector.memset(ones_row[:], 1.0)
                    g_bc = const_pool.tile([P, D], F32)
                    g_row = const_pool.tile([1, D], F32)
                    nc.sync.dma_start(g_row[:], g.ap())
                    for st in range(0, D, FCH):
                        pgb = pss.tile([P, FCH], F32, tag="pgb")
                        nc.tensor.matmul(pgb[:], ones_row[:], g_row[:, st:st + FCH],
                                         start=True, stop=True)
                        nc.scalar.activation(g_bc[:, st:st + FCH], pgb[:],
                                             Act.Copy, bias=0.0, scale=1.0)

                # persistent per-token stats (column layout [P, nT])
                wmeane = stats_pool.tile([P, 1], F32)
                swinv = stats_pool.tile([P, 1], F32)
                rowscale = stats_pool.tile([P, nT], F32)
                s1 = stats_pool.tile([P, nT], F32)
                s2 = stats_pool.tile([P, nT], F32)
                rinv = stats_pool.tile([P, nT], F32)
                aei = stats_pool.tile([P, nT], F32)
                beta = stats_pool.tile([P, nT], F32)
                # stationary pair slab: [d-part, j, slot(qh16|ql), token]
                qpair = qres_pool.tile([P, nD, 2, Tc], FP8)

                # ---- stage A: mean|w| over this core's F/8 slice ----
                acc_a = stats_pool.tile([P, nD], F32)
                for j in range(nD):
                    wa = wa_pool.tile([P, FA], F32)
                    nc.sync.dma_start(wa[:], wA.ap()[j * P:(j + 1) * P, :])
                    wascr = wr_pool.tile([P, max(FG, FA)], F32, tag="wr")
                    nc.scalar.activation(wascr[:, :FA], wa[:], Act.Abs, bias=0.0,
                                         scale=1.0, accum_out=acc_a[:, j:j + 1])
                acc1 = stats_pool.tile([P, 1], F32)
                nc.vector.tensor_reduce(acc1[:], acc_a[:], axis=mybir.AxisListType.X,
                                        op=Alu.add)
                if n_devices > 1 and reps == 1 and not skip_cc:
                    nc.sync.dma_start(cc_in.ap(), acc1[:])
                    nc.gpsimd.collective_compute(
                        "AllReduce", Alu.add,
                        replica_groups=[list(range(n_devices))],
                        ins=[cc_in.ap().opt()],
                        outs=[cc_out.ap().opt()],
                    )
                    ccred = stats_pool.tile([P, 1], F32)
                    nc.sync.dma_start(ccred[:], cc_out.ap())
                else:
                    ccred = acc1
                ptot = pss.tile([P, 1], F32, tag="ptot")
                nc.tensor.matmul(ptot[:], ones_col[:], ccred[:], start=True, stop=True)
                nc.scalar.activation(wmeane[:], ptot[:], Act.Copy,
                                     bias=float(EPS_W), scale=1.0 / float(D * F))
                nc.vector.reciprocal(swinv[:], wmeane[:])

                # ---- stage X: per-token-tile stats + quant + transpose ----
                for i in range(nT):
                    xi = xi_pool.tile([P, D], F32, tag="xi")
                    nc.sync.dma_start(xi[:], x.ap()[i * P:(i + 1) * P, :])
                    if g_is_ones:
                        xuse = xi
                    else:
                        xg = xi_pool.tile([P, D], F32, tag="xg")
                        nc.vector.tensor_tensor(xg[:], xi[:], g_bc[:], op=Alu.mult)
                        xuse = xg
                    nc.vector.tensor_reduce(s2[:, i:i + 1], xuse[:],
                                            axis=mybir.AxisListType.X,
                                            op=Alu.max, apply_absolute_value=True)
                    sqscr = qrow_pool.tile([P, D], BF16, tag="sqscr")
                    nc.scalar.activation(sqscr[:], xi[:], Act.Square, bias=0.0,
                                         scale=1.0, accum_out=s1[:, i:i + 1])
                    # beta chain on [P,1] columns
                    si, zi = s1[:, i:i + 1], s2[:, i:i + 1]
                    ri, ai, bi = rinv[:, i:i + 1], aei[:, i:i + 1], beta[:, i:i + 1]
                    nc.vector.tensor_scalar(si, si, 1.0 / float(D), float(EPS_NORM),
                                            op0=Alu.mult, op1=Alu.add)
                    nc.scalar.activation(si, si, Act.Sqrt, bias=0.0, scale=1.0)
                    nc.vector.reciprocal(ri, si)           # 1/rms
                    nc.vector.tensor_scalar(ai, zi, ri, float(EPS_ACT),
                                            op0=Alu.mult, op1=Alu.add)  # ae
                    nc.vector.tensor_scalar(rowscale[:, i:i + 1], ai,
                                            wmeane[:, 0:1], 1.0 / 127.0,
                                            op0=Alu.mult, op1=Alu.mult)
                    nc.vector.reciprocal(ai, ai)           # 1/ae
                    nc.vector.tensor_scalar(bi, ai, ri, 127.0,
                                            op0=Alu.mult, op1=Alu.mult)  # beta
                    # quant rows: q16 = fp16(CH + x*beta) = CH + q
                    q16 = qrow_pool.tile([P, D], F16, tag="q16")
                    nc.scalar.activation(q16[:], xuse[:], Act.Copy,
                                         bias=CH, scale=bi)
                    qb = qrow_pool.tile([P, D], BF16, tag="qb")
                    nc.vector.tensor_scalar(qb[:], q16[:], CH, None,
                                            op0=Alu.subtract)
                    qht = qrow_pool.tile([P, D], BF16, tag="qht")
                    nc.vector.tensor_scalar(qht[:], qb[:], 1.0 / 16.0, CB,
                                            op0=Alu.mult, op1=Alu.add)
                    qh16 = qrow_pool.tile([P, D], BF16, tag="qh16")
                    nc.vector.tensor_scalar(qh16[:], qht[:], CB, 16.0,
                                            op0=Alu.subtract, op1=Alu.mult)
                    ql = qrow_pool.tile([P, D], BF16, tag="ql")
                    nc.vector.tensor_tensor(ql[:], qb[:], qh16[:], op=Alu.subtract)
                    # transpose 128x128 blocks into PSUM (8 blocks per bank),
                    # drain with fp8 cast into qpair slots
                    for half in range(2):
                        j0 = half * (nD // 2)
                        ph = pst.tile([P, nD // 2, P], BF16, tag="ph")
                        for jj in range(nD // 2):
                            nc.tensor.transpose(
                                ph[:, jj, :],
                                qh16[:, (j0 + jj) * P:(j0 + jj + 1) * P],
                                identb[:])
                        nc.scalar.activation(
                            qpair[:, j0:j0 + nD // 2, 0, i * P:(i + 1) * P],
                            ph[:], Act.Copy, bias=0.0, scale=1.0)
                        pl = pst.tile([P, nD // 2, P], BF16, tag="pl")
                        for jj in range(nD // 2):
                            nc.tensor.transpose(
                                pl[:, jj, :],
                                ql[:, (j0 + jj) * P:(j0 + jj + 1) * P],
                                identb[:])
                        nc.scalar.activation(
                            qpair[:, j0:j0 + nD // 2, 1, i * P:(i + 1) * P],
                            pl[:], Act.Copy, bias=0.0, scale=1.0)

                # ---- stage M: ternarize + DoubleRow matmul + drain ----
                for fg in range(nG):
                    w3 = []
                    for j in range(nD):
                        wb = wb_pool.tile([P, FG], F32)
                        nc.sync.dma_start(wb[:], wT.ap()[j * P:(j + 1) * P,
                                                         fg * FG:(fg + 1) * FG])
                        wr = wr_pool.tile([P, FG], F32, tag="wr")
                        # wr = C + RNE(w*s_w)
                        nc.vector.tensor_scalar(wr[:], wb[:], swinv[:, 0:1], C,
                                                op0=Alu.mult, op1=Alu.add)
                        # clamp in C-space: C + clip(RNE, -1, 1)
                        nc.vector.tensor_scalar(wr[:], wr[:], C - 1.0, C + 1.0,
                                                op0=Alu.max, op1=Alu.min)
                        w3j = w3_pool.tile([P, FG], FP8, tag=f"w3_{j}")
                        nc.scalar.activation(w3j[:], wr[:], Act.Copy,
                                             bias=-C, scale=1.0)
                        w3.append(w3j)
                    for ti in range(nT):
                        pout = pso.tile([P, FG], F32)
                        for j in range(nD):
                            stat = qpair[:, j, :, ti * P:(ti + 1) * P]
                            for fc in range(nCh):
                                mov = w3[j][:, fc * FCH:(fc + 1) * FCH] \
                                    .unsqueeze(1).broadcast_to([P, 2, FCH])
                                nc.tensor.matmul(
                                    pout[:, fc * FCH:(fc + 1) * FCH],
                                    stat, mov,
                                    start=(j == 0), stop=(j == nD - 1),
                                    perf_mode=PerfMode.DoubleRow)
                        ost = osb_pool.tile([P, FG], ODT)
                        nc.scalar.activation(ost[:], pout[:], Act.Copy, bias=0.0,
                                             scale=rowscale[:, ti:ti + 1])
                        nc.sync.dma_start(out.ap()[ti * P:(ti + 1) * P,
                                                   fg * FG:(fg + 1) * FG], ost[:])

                dsb = stats_pool.tile([1, 8], F32)
                nc.vector.memset(dsb[:], 1.0)
                nc.sync.dma_start(done.ap(), dsb[:])

            if reps == 1:
                _kbody()
            else:
                with tc.For_i(0, reps, 1):
                    _kbody()

    if not os.environ.get("BITLIN_NO_STRIP"):
        _strip_redundant_ldweights(nc)
    nc.compile()
    return nc


_prog_cache = {}


def _get_program(Tc, D, F, g_is_ones=True, out_f32=False):
    key = (Tc, D, F, g_is_ones, out_f32)
    if key not in _prog_cache:
        _prog_cache[key] = build_program(Tc, D, F, g_is_ones=g_is_ones,
                                         out_f32=out_f32)
    return _prog_cache[key]


def make_in_maps(x, norm_weight, weight):
    B, S, D = x.shape
    F = weight.shape[0]
    T = B * S
    Tc = T // NCORES
    xf = np.ascontiguousarray(x.reshape(T, D), dtype=np.float32)
    wTv = np.ascontiguousarray(weight.T).astype(np.float32, copy=False)
    gv = np.ascontiguousarray(norm_weight.reshape(1, D), dtype=np.float32)
    FA = F // NCORES
    in_maps = []
    for c in range(NCORES):
        in_maps.append({
            "x": xf[c * Tc:(c + 1) * Tc],
            "wT": wTv,
            "wA": np.ascontiguousarray(wTv[:, c * FA:(c + 1) * FA]),
            "g": gv,
        })
    return in_maps, (B, S, T, Tc, D, F)


def kernel(x, norm_weight, weight):
    x = np.asarray(x)
    norm_weight = np.asarray(norm_weight)
    weight = np.asarray(weight)
    in_maps, (B, S, T, Tc, D, F) = make_in_maps(x, norm_weight, weight)
    out_f32 = bool(os.environ.get("BITLIN_OUT_F32"))
    nc = _get_program(Tc, D, F, g_is_ones=bool(np.all(norm_weight == 1.0)),
                      out_f32=out_f32)
    res = None
    last_err = None
    for _ in range(3):
        try:
            res = run_bass_kernel_spmd(nc, in_maps, core_ids=list(range(NCORES)))
            break
        except Exception as e:
            last_err = e
    if res is None:
        raise last_err
    outp = np.concatenate([np.asarray(res.results[c]["out"], dtype=np.float32)
                           for c in range(NCORES)], axis=0)
    return np.ascontiguousarray(outp.reshape(B, S, F))
